# revision 11
# baseline (speedup 1.0000x reference)
"""BiLevelRoutingAttention Trainium2 kernel (v2).

Strategy (8 NeuronCores, data-parallel over batch: 2 batches/core, 32 (b,t)
tiles per core). All layouts feature-major ("T-layout"): qT/kT = W^T x^T
(bf16 matmuls, fp32 PSUM), V token-major.

v2 restructure vs v1 (1.07 ms):
  - Routing hoisted per batch: region features (exact fp32 window-sum
    matmuls), sim (tiny fp32 MMs), top-4 threshold via MAX8 + batched
    is_ge, additive window mask built once for all 16 t.
  - Z computed pre-broadcast: all-ones [128,32] stationary per col-group
    replicates each head's softmax denominator onto its 32 d-partitions
    directly in PSUM (kills the M=1 ones-matmuls + DRAM bounce broadcast).
  - 1/Z via single-pass reciprocal_approx_fast on DVE (the iterative
    nc.vector.reciprocal was 4 us per tile).
  - Per-tile fp32 routing matmuls (400 ns LDWEIGHTS pairs) removed from
    the tile loop entirely.
"""

import sys

sys.path.insert(0, "/opt/trn_rl_repo")

import numpy as np
import ml_dtypes

import concourse.bass as bass
import concourse.bacc as bacc
import concourse.mybir as mybir
import concourse.tile as tile
from concourse.bass_utils import run_bass_kernel_spmd

BF16 = mybir.dt.bfloat16
F32 = mybir.dt.float32

NCORES = 8
B, T, S, C = 16, 16, 256, 256
NW, WIN, NH, D, TK = 8, 32, 8, 32, 4
BPC = B // NCORES  # batches per core
SCALE = float(D) ** -0.5
MASKVAL = -1e9

_CACHE = {}


def _build_nc(nt=T):
    nc = bacc.Bacc("TRN2", target_bir_lowering=False, debug=False)
    AL = mybir.AluOpType
    ACTF = mybir.ActivationFunctionType

    xt_d = nc.dram_tensor("xt", [BPC, nt, C, S], BF16, kind="ExternalInput")
    xs_d = nc.dram_tensor("xsumt", [BPC, C, nt, NW], F32, kind="ExternalInput")
    wqk_d = nc.dram_tensor("wqk_bf", [C, 2 * C], BF16, kind="ExternalInput")
    wqkf_d = nc.dram_tensor("wqk_f32", [C, 2 * C], F32, kind="ExternalInput")
    wv_d = nc.dram_tensor("wv_bf", [C, C], BF16, kind="ExternalInput")
    wp_d = nc.dram_tensor("wproj_bf", [C, C], BF16, kind="ExternalInput")
    bqk_d = nc.dram_tensor("bqk_cols", [128, 4], F32, kind="ExternalInput")
    bqkr_d = nc.dram_tensor("bqk_reg", [128, 4], F32, kind="ExternalInput")
    bvbf_d = nc.dram_tensor("bv_bf", [1, C], BF16, kind="ExternalInput")
    bp_d = nc.dram_tensor("bproj_row", [1, C], F32, kind="ExternalInput")
    e8r_d = nc.dram_tensor("e8r", [128, S], BF16, kind="ExternalInput")
    out_d = nc.dram_tensor("out", [BPC, nt, 2, 128, C], F32, kind="ExternalOutput")

    with tile.TileContext(nc) as tc:
        with (
            tc.tile_pool(name="wpool", bufs=1) as wp,
            tc.tile_pool(name="bpool", bufs=2) as bp,
            tc.tile_pool(name="xpool", bufs=6) as xp,
            tc.tile_pool(name="mid", bufs=4) as mp,
            tc.tile_pool(name="msk", bufs=4) as kp,
            tc.tile_pool(name="exps", bufs=3) as ep,
            tc.tile_pool(name="b1", bufs=1, space="PSUM") as pb1,
            tc.tile_pool(name="sc", bufs=2, space="PSUM") as psc,
        ):
            # ---- weights / constants (loaded once) ----
            wqk_sb = wp.tile([128, 2, 2 * C], BF16)
            nc.sync.dma_start(out=wqk_sb, in_=wqk_d.ap().rearrange("(cc p) j -> p cc j", p=128))
            wqkf_sb = wp.tile([128, 2, 2 * C], F32)
            nc.sync.dma_start(out=wqkf_sb, in_=wqkf_d.ap().rearrange("(cc p) j -> p cc j", p=128))
            wv_sb = wp.tile([128, 2, C], BF16)
            nc.sync.dma_start(out=wv_sb, in_=wv_d.ap().rearrange("(cc p) j -> p cc j", p=128))
            wp_sb = wp.tile([128, 2, C], BF16)
            nc.sync.dma_start(out=wp_sb, in_=wp_d.ap().rearrange("(cc p) j -> p cc j", p=128))
            bqk_sb = wp.tile([128, 4], F32)
            nc.sync.dma_start(out=bqk_sb, in_=bqk_d.ap())
            bqkr_sb = wp.tile([128, 4], F32)
            nc.sync.dma_start(out=bqkr_sb, in_=bqkr_d.ap())
            # proj bias row pre-broadcast to all 128 partitions (DMA supports
            # partition-step-0 source APs; DVE does not)
            bp_sb = wp.tile([128, C], F32)
            nc.sync.dma_start(out=bp_sb, in_=bp_d.ap().to_broadcast([128, C]))
            e8r_sb = wp.tile([128, S], BF16)
            nc.sync.dma_start(out=e8r_sb, in_=e8r_d.ap())
            ones32_sb = wp.tile([128, 32], BF16)
            nc.vector.memset(ones32_sb, 1.0)
            onesr_sb = wp.tile([1, 128], BF16)
            nc.vector.memset(onesr_sb, 1.0)
            bvr_sb = wp.tile([1, C], BF16)
            nc.sync.dma_start(out=bvr_sb, in_=bvbf_d.ap())

            mw_sbs = []
            for b in range(BPC):
                # ================= batch preamble: routing =================
                xs_sb = bp.tile([128, 2, nt * NW], F32, tag="xsb")
                nc.sync.dma_start(
                    out=xs_sb,
                    in_=xs_d[b].rearrange("(cc p) t n -> p cc (t n)", p=128))

                # region features: [q;k]^T = Wqk^T @ xsum  (exact fp32)
                rs_ps = pb1.tile([128, 4, nt * NW], F32, tag="zrep")
                for jb in range(4):
                    for cc in range(2):
                        nc.tensor.matmul(rs_ps[:, jb, :],
                                         lhsT=wqkf_sb[:, cc, jb * 128:(jb + 1) * 128],
                                         rhs=xs_sb[:, cc, :],
                                         start=(jb == 0 and cc == 0),
                                         stop=(jb == 3 and cc == 1))
                rgs_sb = bp.tile([128, 4, nt * NW], F32, tag="rgs")
                nc.vector.tensor_tensor(
                    out=rgs_sb, in0=rs_ps,
                    in1=bqkr_sb[:].unsqueeze(-1).to_broadcast([128, 4, nt * NW]),
                    op=AL.add)

                # sim: per (t, head) 8x8 region-similarity, fp32 tiny MMs on
                # diagonal 32x32 array tiles
                sim_ps = pb1.tile([128, 2, nt * NW], F32, tag="zrep")
                nc.vector.memset(sim_ps, 0.0)
                for t in range(nt):
                    for jbq in range(2):
                        for rg in range(4):
                            nc.tensor.matmul(
                                sim_ps[32 * rg:32 * rg + 8, jbq,
                                       t * NW:(t + 1) * NW],
                                lhsT=rgs_sb[32 * rg:32 * rg + 32, jbq,
                                            t * NW:(t + 1) * NW],
                                rhs=rgs_sb[32 * rg:32 * rg + 32, 2 + jbq,
                                           t * NW:(t + 1) * NW],
                                start=False, stop=False,
                                skip_group_check=True,
                                tile_position=(32 * rg, 32 * rg))
                sim_sb = bp.tile([128, 2, nt * NW], F32, tag="sim")
                nc.vector.tensor_copy(out=sim_sb, in_=sim_ps)

                # top-4 threshold per (head, qwin): 4th largest of the 8 sims
                mx_sb = bp.tile([128, 2, nt * NW], F32, tag="mx")
                for t in range(nt):
                    for jbq in range(2):
                        nc.vector.max(out=mx_sb[:, jbq, t * NW:(t + 1) * NW],
                                      in_=sim_sb[:, jbq, t * NW:(t + 1) * NW])
                mw_sb = bp.tile([128, 2, nt * NW], BF16, tag="mw")
                for jbq in range(2):
                    nc.vector.tensor_tensor(
                        out=mw_sb[:, jbq, :].rearrange("p (t n) -> p t n", n=NW),
                        in0=sim_sb[:, jbq, :].rearrange("p (t n) -> p t n", n=NW),
                        in1=mx_sb[:, jbq, 3::NW].unsqueeze(-1)
                            .to_broadcast([128, nt, NW]),
                        op=AL.is_ge)
                # mask = (m01 - 1) * 1e9  ->  0 selected / -1e9 dropped
                nc.vector.tensor_scalar(out=mw_sb, in0=mw_sb,
                                        scalar1=1.0, scalar2=-MASKVAL,
                                        op0=AL.subtract, op1=AL.mult)
                mw_sbs.append(mw_sb)

            # ================= per-tile main loop =================
            # batches interleaved: doubles the pool of independent ready work
            # at every point so the PE never starves during exp chains
            for t in range(nt):
                for b in range(BPC):
                    mw_sb = mw_sbs[b]
                    # window-expand mask: one jbq on gpsimd, one on DVE so the
                    # expanded mask is ready well before the score matmuls
                    mwx_sb = kp.tile([128, 2, S], BF16, tag="mwx", bufs=6)
                    nc.gpsimd.tensor_copy(
                        out=mwx_sb[:, 0, :],
                        in_=mw_sb[:, 0, t * NW:(t + 1) * NW]
                            .unsqueeze(-1).to_broadcast([128, NW, WIN]))
                    nc.vector.tensor_copy(
                        out=mwx_sb[:, 1, :],
                        in_=mw_sb[:, 1, t * NW:(t + 1) * NW]
                            .unsqueeze(-1).to_broadcast([128, NW, WIN]))

                    xt_sb = xp.tile([128, 2, S], BF16, tag="xt")
                    nc.sync.dma_start(
                        out=xt_sb,
                        in_=xt_d[b, t].rearrange("(cc p) s -> p cc s", p=128))

                    # ---- qT / kT (feature-major) ----
                    qk_sb = mp.tile([128, 4, S], BF16, tag="qk")
                    for half in range(2):
                        qps = pb1.tile([128, 2, S], F32, tag="qps")
                        for j in range(2):
                            jb = 2 * half + j
                            for cc in range(2):
                                nc.tensor.matmul(
                                    qps[:, j, :],
                                    lhsT=wqk_sb[:, cc, jb * 128:(jb + 1) * 128],
                                    rhs=xt_sb[:, cc, :],
                                    start=(j == 0 and cc == 0),
                                    stop=(j == 1 and cc == 1))
                        nc.vector.tensor_tensor(
                            out=qk_sb[:, 2 * half:2 * half + 2, :], in0=qps,
                            in1=bqk_sb[:, 2 * half:2 * half + 2].unsqueeze(-1)
                                .to_broadcast([128, 2, S]),
                            op=AL.add)

                    # ---- V (token-major), copy on ACT ----
                    v_sb = mp.tile([128, 2, C], BF16, tag="v")
                    vps = pb1.tile([128, 2, C], F32, tag="vps")
                    for sb_ in range(2):
                        for cc in range(2):
                            nc.tensor.matmul(vps[:, sb_, :],
                                             lhsT=xt_sb[:, cc, sb_ * 128:(sb_ + 1) * 128],
                                             rhs=wv_sb[:, cc, :],
                                             start=(sb_ == 0 and cc == 0), stop=False)
                        nc.tensor.matmul(vps[:, sb_, :], lhsT=onesr_sb, rhs=bvr_sb,
                                         start=False, stop=(sb_ == 1))
                    nc.vector.tensor_copy(out=v_sb, in_=vps)

                    # ---- scores^T + mask, exp ----
                    # per-(jbq, rg-pair) PSUM tiles: each rg owns a full bank
                    # (concurrent row-group matmuls must write different PSUM
                    # banks), double-buffered so the next pair's score matmuls
                    # overlap this pair's exp
                    expT = ep.tile([128, 2, 4, 2 * S], BF16, tag="expT")
                    for jbq in range(2):
                        for rp in range(2):
                            sc_ps = psc.tile([128, 2, 2 * S], F32, tag="sc")
                            for rr in range(2):
                                rg = 2 * rp + rr
                                for kb in range(2):
                                    nc.tensor.matmul(
                                        sc_ps[:, rr, kb * S:(kb + 1) * S],
                                        lhsT=qk_sb[32 * rg:32 * rg + 32, 2 + jbq,
                                                   kb * 128:(kb + 1) * 128],
                                        rhs=qk_sb[32 * rg:32 * rg + 32, jbq, :],
                                        start=(kb == 0), stop=False,
                                        skip_group_check=True,
                                        tile_position=(32 * rg, 0))
                                    nc.tensor.matmul(
                                        sc_ps[:, rr, kb * S:(kb + 1) * S],
                                        lhsT=mwx_sb[32 * rg:32 * rg + 8, jbq,
                                                    kb * 128:(kb + 1) * 128],
                                        rhs=e8r_sb[32 * rg:32 * rg + 8, :],
                                        start=False, stop=(kb == 1),
                                        skip_group_check=True,
                                        tile_position=(32 * rg, 0))
                            nc.scalar.activation(
                                out=expT[:, jbq, 2 * rp:2 * rp + 2, :],
                                in_=sc_ps, func=ACTF.Exp, scale=SCALE)

                    # ---- Z, pre-broadcast onto each head's 32 partitions ----
                    zrep = pb1.tile([128, 2, S], F32, tag="zrep")
                    for jbq in range(2):
                        for rg in range(4):
                            for kb in range(2):
                                nc.tensor.matmul(
                                    zrep[32 * rg:32 * rg + 32, jbq, :],
                                    lhsT=ones32_sb,
                                    rhs=expT[:, jbq, rg, kb * S:(kb + 1) * S],
                                    start=(jbq == 0 and kb == 0),
                                    stop=(jbq == 1 and kb == 1),
                                    skip_group_check=True,
                                    tile_position=(0, 32 * rg))
                    zinv_sb = mp.tile([128, 2, S], F32, tag="zinv")
                    nc.vector.reciprocal_approx_fast(out=zinv_sb, in_=zrep)

                    # ---- PV (col-packed, both quads in one bank) ----
                    at = pb1.tile([128, 2, S], F32, tag="atpo")
                    for jbq in range(2):
                        for rg in range(4):
                            hh = 4 * jbq + rg
                            for kb in range(2):
                                nc.tensor.matmul(
                                    at[32 * rg:32 * rg + 32, jbq, :],
                                    lhsT=v_sb[:, kb, 32 * hh:32 * hh + 32],
                                    rhs=expT[:, jbq, rg, kb * S:(kb + 1) * S],
                                    start=(jbq == 0 and kb == 0),
                                    stop=(jbq == 1 and kb == 1),
                                    skip_group_check=True,
                                    tile_position=(0, 32 * rg))
                    atn_sb = mp.tile([128, 2, S], BF16, tag="atn")
                    nc.vector.tensor_tensor(out=atn_sb, in0=at, in1=zinv_sb,
                                            op=AL.mult)

                    # ---- out projection ----
                    out_sb = mp.tile([128, 2, C], F32, tag="out")
                    po = pb1.tile([128, 2, C], F32, tag="atpo")
                    for sb_ in range(2):
                        for cc in range(2):
                            nc.tensor.matmul(po[:, sb_, :],
                                             lhsT=atn_sb[:, cc, sb_ * 128:(sb_ + 1) * 128],
                                             rhs=wp_sb[:, cc, :],
                                             start=(sb_ == 0 and cc == 0),
                                             stop=(sb_ == 1 and cc == 1))
                    nc.vector.tensor_tensor(
                        out=out_sb, in0=po,
                        in1=bp_sb[:].unsqueeze(1).to_broadcast([128, 2, C]),
                        op=AL.add)
                    nc.sync.dma_start(out=out_d[b, t].rearrange("s p c -> p s c"),
                                      in_=out_sb)

    nc.compile()
    return nc


def _host_prep(x, w_qkv, b_qkv, w_proj, b_proj):
    bf16 = ml_dtypes.bfloat16
    x4 = x.reshape(B, T, S, C)
    xt = np.ascontiguousarray(x4.transpose(0, 1, 3, 2)).astype(bf16)
    xsum = x4.reshape(B, T, NW, WIN, C).sum(3, dtype=np.float64).astype(np.float32)
    xsumt = np.ascontiguousarray(xsum.transpose(0, 3, 1, 2))  # [B, C, T, NW]

    shared = {
        "wqk_bf": np.ascontiguousarray(w_qkv[:, :2 * C]).astype(bf16),
        "wqk_f32": np.ascontiguousarray(w_qkv[:, :2 * C]).astype(np.float32),
        "wv_bf": np.ascontiguousarray(w_qkv[:, 2 * C:]).astype(bf16),
        "wproj_bf": w_proj.astype(bf16),
        "bqk_cols": np.ascontiguousarray(
            b_qkv[:2 * C].reshape(4, 128).T).astype(np.float32),
        "bqk_reg": np.ascontiguousarray(
            (WIN * b_qkv[:2 * C]).reshape(4, 128).T).astype(np.float32),
        "bv_bf": b_qkv[2 * C:].reshape(1, C).astype(bf16),
        "bproj_row": b_proj.reshape(1, C).astype(np.float32),
        "e8r": _make_e8r(),
    }
    in_maps = []
    for core in range(NCORES):
        b0 = core * BPC
        m = dict(shared)
        m["xt"] = np.ascontiguousarray(xt[b0:b0 + BPC])
        m["xsumt"] = np.ascontiguousarray(xsumt[b0:b0 + BPC])
        in_maps.append(m)
    return in_maps


def _make_e8r():
    e = np.zeros((128, S), ml_dtypes.bfloat16)
    q = np.arange(S) // WIN  # query window of column q
    for rg in range(4):
        for n in range(NW):
            e[32 * rg + n, q == n] = 1.0
    return e


def kernel(x, w_qkv, b_qkv, w_proj, b_proj, **_unused_scalars):
    x = np.asarray(x, dtype=np.float32)
    w_qkv = np.asarray(w_qkv, dtype=np.float32)
    b_qkv = np.asarray(b_qkv, dtype=np.float32)
    w_proj = np.asarray(w_proj, dtype=np.float32)
    b_proj = np.asarray(b_proj, dtype=np.float32)

    if "nc" not in _CACHE:
        _CACHE["nc"] = _build_nc()
    nc = _CACHE["nc"]

    in_maps = _host_prep(x, w_qkv, b_qkv, w_proj, b_proj)
    res = run_bass_kernel_spmd(nc, in_maps, core_ids=list(range(NCORES)))

    out = np.empty((B, T, 2, 128, C), np.float32)
    for core in range(NCORES):
        out[core * BPC:(core + 1) * BPC] = res.results[core]["out"]
    # [B, T, sb, p, C] -> [B, T*S, C]
    return out.reshape(B, T * S, C)
